# revision 10
# baseline (speedup 1.0000x reference)
"""Trainium2 Bass kernel for masked attention scoring (sparse_attention).

Computes, per batch b:
    proj = y @ M^T                      # [B, D]
    eij  = tanh(einsum('bsd,bd->bs', x, proj))
    a    = exp(eij) * mask
    a    = a / (sum_s a + EPS)

Sharding: data-parallel over batch B=32 across 8 NeuronCores (4 batches
per core). M is replicated; all reductions stay local per shard.

Design (memory-bound; the x f16 stream at ~420 GB/s/core is the
roofline; instruction fetch rides DMA engine 0, so PE-instruction
bytes executed during the stream directly slow that engine and delay
every DMA-completion semaphore -> keep instruction count low):
  - x splits into a host-TRANSPOSED part xt (s-chunks 0..NT-1, [e, s]
    with d-rows pairwise interleaved, 5.6 KB descriptors) consumed by
    TensorE as 128x128 LDWEIGHTS tiles against a projT column (eij
    accumulates in PSUM on 128 partitions), and a natural-layout part
    xn (chunks NT..15, host-packed so each partition row is one 10 KB
    descriptor) for the cheap-in-instructions DVE STT / DVE-mul + ACT
    Copy-accum path.
  - The PE x-accumulation runs per batch in two phases (one per e-chunk
    half / DMA piece), each phase a contiguous 4-matmul PSUM group per
    column in its own bank; ACT copy + single-PSUM-operand DVE add
    merge the banks.
  - Per-batch epilogue runs INLINE (tanh+exp, fused DVE mask-mul +
    row-accum -> au / cs), then sum+broadcast in ONE matmul
    (lhsT=ones[P,J] replicates the total onto J partitions), +eps,
    reciprocal, PE transpose of unnormalized au, normalization as a
    per-partition ACT scale on the final copy.  Out-DMAs ride the
    second HWDGE ring (scalar engine) so they skip the x backlog on
    the sync-ring FIFO.
  - The last batch's phase-B xt piece and epilogue are split at
    j=JSPLIT so nearly all tail work completes before the final
    ~400 KB piece lands; mask/M/y/xn stream before xt so the last
    batch's dependent chain is the shortest one.
"""

import os
import sys

import numpy as np

for _p in ("/opt/trn_rl_repo",):
    if os.path.isdir(_p) and _p not in sys.path:
        sys.path.insert(0, _p)

B, S, D = 32, 2048, 1024
NCORES = 8
BL = B // NCORES        # batches per core
P = 128                 # SBUF partitions
J = S // P              # 16 s-chunks of 128 per batch
DC = D // P             # 8 e-chunks of 128
NT = 11                 # PE-path s-chunks per batch (j = 0..NT-1)
NN = J - NT             # natural-path s-chunks (j = NT..J-1)
N_STT = 2               # of NN: chunks done as fused STT on DVE
ST = NT * P
SN = NN * P
JSPLIT = 8              # last batch: phase-B piece/epilogue split (j)
EPS = 1e-7

_CACHE = {}


def _build():
    import concourse.bacc as bacc
    import concourse.bass as bass_mod
    import concourse.tile as tile
    from concourse import mybir
    from concourse.masks import make_identity

    f32 = mybir.dt.float32
    f16 = mybir.dt.float16

    nc = bacc.Bacc("TRN2", target_bir_lowering=False, debug=False,
                   num_devices=NCORES)

    xt_ext = nc.dram_tensor("xt16", [BL, D, ST], f16, kind="ExternalInput").ap()
    xn_ext = nc.dram_tensor("xn16", [BL, P, NN * D], f16,
                            kind="ExternalInput").ap()
    y_ext = nc.dram_tensor("yT16", [D, BL], f16, kind="ExternalInput").ap()
    m_ext = nc.dram_tensor("MT16", [D, D], f16, kind="ExternalInput").ap()
    mk_ext = nc.dram_tensor("maskT", [P, BL, J], f32, kind="ExternalInput").ap()
    out_ext = nc.dram_tensor("out", [BL, S], f32, kind="ExternalOutput").ap()

    with tile.TileContext(nc) as tc:
        with (
            tc.tile_pool(name="consts", bufs=1) as consts,
            tc.tile_pool(name="psum_proj", bufs=1, space="PSUM") as psum_proj,
            tc.tile_pool(name="psum_eij", bufs=1, space="PSUM") as psum_eij,
            tc.tile_pool(name="psum_eij2", bufs=1, space="PSUM") as psum_eij2,
            tc.tile_pool(name="psum_pb", bufs=1, space="PSUM") as psum_pb,
            tc.tile_pool(name="psum_small", bufs=1, space="PSUM") as psum_small,
            tc.tile_pool(name="scr", bufs=6) as scr_pool,
        ):
            identity16 = consts.tile([P, P], f16)
            make_identity(nc, identity16)
            identity32 = consts.tile([P, P], f32)
            make_identity(nc, identity32)
            ones_pj = consts.tile([P, J], f32)
            nc.vector.memset(ones_pj, 1.0)

            # ---- M^T ships pre-transposed + pairwise row-interleaved:
            # host row q=dc2*256+2p+t holds MT row d=dc2*256+t*128+p, so
            # each (p,dc2) partition line is a 4KB contiguous run and the
            # contraction mapping stays d = dc*128+p with dc=2*dc2+t.
            mtsb = consts.tile([P, DC // 2, 2, D], f16)
            m_src = m_ext.rearrange("(dc2 p two) e -> p dc2 two e",
                                    p=P, two=2)
            nc.sync.dma_start(out=mtsb, in_=m_src)

            # warm the PE clock (1.2 -> 2.4 GHz needs ~3us sustained)
            warm_ps = psum_small.tile([P, P], f16, tag="small")
            for _ in range(12):
                nc.tensor.transpose(warm_ps, identity16, identity16)

            # ---- small inputs ----
            yT = consts.tile([P, DC, BL], f16)
            nc.sync.dma_start(
                out=yT, in_=y_ext.rearrange("(dc p) b -> p dc b", p=P))
            mask_all = consts.tile([P, BL, J], f32)
            nc.sync.dma_start(out=mask_all, in_=mk_ext)

            # ---- x DMAs: xn (slow consumers) first, xt last ----
            xt_tiles = []
            xn_tiles = []
            for b in range(BL):
                xt_tiles.append(consts.tile([P, DC // 2, 2, ST], f16,
                                            name=f"xt{b}"))
                xn_tiles.append(consts.tile([P, NN, D], f16, name=f"xn{b}"))
            for b in range(BL):
                # host-packed: each partition row is one 10 KB run
                nc.sync.dma_start(
                    out=xn_tiles[b],
                    in_=xn_ext[b].rearrange("p (i e) -> p i e", i=NN))
            # xt rows are 2 interleaved d-rows = 5.6KB descriptors; split
            # by e-chunk halves (phases); the last batch's phase-B piece
            # additionally splits at j=JSPLIT so the final piece is small
            for b in range(BL):
                xt_src = xt_ext[b].rearrange("(dc2 p two) s -> p dc2 two s",
                                             p=P, two=2)
                nc.sync.dma_start(out=xt_tiles[b][:, 0:2, :, :],
                                  in_=xt_src[:, 0:2, :, :])
                if b < BL - 1:
                    nc.sync.dma_start(out=xt_tiles[b][:, 2:, :, :],
                                      in_=xt_src[:, 2:, :, :])
                else:
                    for dc2 in (2, 3):
                        nc.sync.dma_start(
                            out=xt_tiles[b][:, dc2:dc2 + 1, :,
                                            0:JSPLIT * P],
                            in_=xt_src[:, dc2:dc2 + 1, :, 0:JSPLIT * P])
                    for dc2 in (2, 3):
                        nc.sync.dma_start(
                            out=xt_tiles[b][:, dc2:dc2 + 1, :,
                                            JSPLIT * P:],
                            in_=xt_src[:, dc2:dc2 + 1, :, JSPLIT * P:])

            # ---- proj[b, e] = sum_d y[b, d] * M[e, d]  (PSUM f32) ----
            proj_ps = psum_proj.tile([BL, D], f32)
            for dc in range(DC):
                for eh in range(2):
                    nc.tensor.matmul(
                        proj_ps[:, eh * 512:(eh + 1) * 512],
                        lhsT=yT[:, dc, :],
                        rhs=mtsb[:, dc // 2, dc % 2,
                                 eh * 512:(eh + 1) * 512],
                        start=(dc == 0),
                        stop=(dc == DC - 1),
                    )
            proj_sb = consts.tile([BL, D], f16)
            nc.scalar.copy(proj_sb[:, 0:512], proj_ps[:, 0:512])
            nc.scalar.copy(proj_sb[:, 512:], proj_ps[:, 512:])

            # ---- projT[p, ec, b] = proj[b, ec*128+p] via PE transposes ----
            projT = consts.tile([P, DC, BL], f16)
            for ec in range(DC):
                tp_ps = psum_small.tile([P, BL], f16, tag="ptr", bufs=1)
                nc.tensor.transpose(
                    tp_ps, proj_sb[:, ec * P:(ec + 1) * P],
                    identity16[:BL, :BL])
                nc.scalar.copy(projT[:, ec, :], tp_ps)

            # ---- broadcast proj rows across partitions (DVE path) ----
            projbc = []
            for b in range(BL):
                sel = consts.tile([BL, P], f16, name=f"sel{b}")
                nc.gpsimd.memset(sel, 0.0)
                nc.gpsimd.affine_select(
                    out=sel, in_=sel,
                    compare_op=mybir.AluOpType.not_equal,
                    fill=1.0, base=-b,
                    pattern=[[0, P]], channel_multiplier=1)
                pb = consts.tile([P, D], f16, name=f"projbc{b}")
                for eh in range(2):
                    pb_ps = psum_pb.tile([P, 512], f32, tag="pbps")
                    nc.tensor.matmul(
                        pb_ps,
                        lhsT=sel,
                        rhs=proj_sb[:, eh * 512:(eh + 1) * 512],
                        start=True, stop=True)
                    if eh == 0:
                        nc.vector.tensor_copy(pb[:, eh * 512:(eh + 1) * 512],
                                              pb_ps)
                    else:
                        nc.scalar.copy(pb[:, eh * 512:(eh + 1) * 512], pb_ps)
                projbc.append(pb)

            # ---- main pass tiles ----
            eij_a = consts.tile([P, BL, NT], f32)
            eij_all = consts.tile([P, BL, J], f32)
            au = consts.tile([P, BL, J], f32)
            cs = consts.tile([P, BL], f32)
            csb = consts.tile([P, 1], f32)
            th = consts.tile([P, BL, J], f32)
            ex = consts.tile([P, BL, J], f32)

            # ---- main pass: DVE/ACT on xn chunks (writes eij_all[NT:]) ----
            for b in range(BL):
                xn = xn_tiles[b]
                for i in range(N_STT):
                    scr = scr_pool.tile([P, D], f16, tag="scr")
                    nc.vector.scalar_tensor_tensor(
                        out=scr,
                        in0=xn[:, i, :],
                        scalar=1.0,
                        in1=projbc[b],
                        op0=mybir.AluOpType.mult,
                        op1=mybir.AluOpType.mult,
                        accum_out=eij_all[:, b, NT + i:NT + i + 1],
                    )
                i = N_STT
                while i < NN:
                    w = min(2, NN - i)
                    scr2 = scr_pool.tile([P, 2, D], f16, tag="scr2", bufs=4)
                    pbc2 = bass_mod.AP(
                        tensor=projbc[b].tensor,
                        offset=projbc[b].offset,
                        ap=[projbc[b].ap[0], [0, w]] + projbc[b].ap[1:])
                    nc.vector.tensor_mul(scr2[:, 0:w, :],
                                         xn[:, i:i + w, :], pbc2)
                    for k in range(w):
                        dump = scr_pool.tile([P, D], f16, tag="dump", bufs=4)
                        nc.scalar.activation(
                            dump, scr2[:, k, :],
                            mybir.ActivationFunctionType.Copy,
                            accum_out=eij_all[:, b, NT + i + k:NT + i + k + 1])
                    i += w

            # ---- per-batch epilogue pieces ----
            def epilogue(b, j0, j1, m0, m1, ps2, cs_col):
                # merge PE phase banks for columns m0..m1, then
                # tanh/exp/mask-mul+accum over columns j0..j1
                if m1 > m0:
                    nc.vector.tensor_add(eij_all[:, b, m0:m1],
                                         eij_a[:, b, m0:m1], ps2[:, b, m0:m1])
                nc.scalar.activation(th[:, b, j0:j1], eij_all[:, b, j0:j1],
                                     mybir.ActivationFunctionType.Tanh)
                nc.scalar.activation(ex[:, b, j0:j1], th[:, b, j0:j1],
                                     mybir.ActivationFunctionType.Exp)
                nc.vector.scalar_tensor_tensor(
                    out=au[:, b, j0:j1],
                    in0=ex[:, b, j0:j1],
                    scalar=1.0,
                    in1=mask_all[:, b, j0:j1],
                    op0=mybir.AluOpType.mult,
                    op1=mybir.AluOpType.mult,
                    accum_out=cs_col,
                )

            def finishing(b, cs_cols):
                # total-sum replicated onto J partitions in one matmul
                tot_ps = psum_small.tile([J, 1], f32, tag="small")
                for i, col in enumerate(cs_cols):
                    nc.tensor.matmul(tot_ps, lhsT=ones_pj, rhs=col,
                                     start=(i == 0),
                                     stop=(i == len(cs_cols) - 1))
                at_ps = psum_small.tile([J, P], f32, tag="attr")
                nc.tensor.transpose(at_ps, au[:, b, :], identity32)
                rec = consts.tile([J, 1], f32, name=f"rec{b}")
                nc.vector.tensor_scalar_add(rec, tot_ps, EPS)
                nc.vector.reciprocal(rec, rec)
                an_t = consts.tile([J, P], f32, name=f"ant{b}")
                nc.scalar.activation(an_t, at_ps,
                                     mybir.ActivationFunctionType.Copy,
                                     scale=rec)
                # out-DMAs ride the scalar HWDGE ring: independent FIFO,
                # and the issue directly follows the scale-copy in the
                # ACT queue (no cross-engine hop).
                nc.scalar.dma_start(
                    out=out_ext[b].rearrange("(j p) -> j p", p=P),
                    in_=an_t)

            # ---- main pass: PE on xt chunks; epilogue inline ----
            for b in range(BL):
                xt = xt_tiles[b]
                eij_ps = psum_eij.tile([P, BL, NT], f32, tag="eijA")
                eij_ps2 = psum_eij2.tile([P, BL, NT], f32, tag="eijB")
                # phase A (e-chunks 0..3), bank A; contiguous group per col
                for j in range(NT):
                    for ec in range(4):
                        nc.tensor.matmul(
                            eij_ps[:, b, j:j + 1],
                            lhsT=xt[:, ec // 2, ec % 2, j * P:(j + 1) * P],
                            rhs=projT[:, ec, b:b + 1],
                            start=(ec == 0),
                            stop=(ec == 3),
                        )
                nc.scalar.copy(eij_a[:, b, :], eij_ps[:, b, :])

                def phase_b(j0, j1):
                    for j in range(j0, j1):
                        for ec in range(4, 8):
                            nc.tensor.matmul(
                                eij_ps2[:, b, j:j + 1],
                                lhsT=xt[:, ec // 2, ec % 2,
                                        j * P:(j + 1) * P],
                                rhs=projT[:, ec, b:b + 1],
                                start=(ec == 4),
                                stop=(ec == 7),
                            )

                if b < BL - 1:
                    phase_b(0, NT)
                    epilogue(b, 0, J, 0, NT, eij_ps2, cs[:, b:b + 1])
                    finishing(b, [cs[:, b:b + 1]])
                else:
                    phase_b(0, JSPLIT)
                    epilogue(b, 0, JSPLIT, 0, JSPLIT, eij_ps2,
                             cs[:, b:b + 1])
                    phase_b(JSPLIT, NT)
                    epilogue(b, JSPLIT, J, JSPLIT, NT, eij_ps2, csb)
                    finishing(b, [cs[:, b:b + 1], csb])

    nc.compile()
    return nc


def _get_nc():
    if "nc" not in _CACHE:
        _CACHE["nc"] = _build()
    return _CACHE["nc"]


def _in_maps(x, y, mask, M):
    x16 = np.asarray(x, dtype=np.float32).astype(np.float16)
    y16 = np.asarray(y, dtype=np.float32).astype(np.float16)
    MT16 = np.asarray(M, dtype=np.float32).astype(np.float16).T
    # interleave d-rows pairwise: row q=dc2*256+2p+t holds d=dc2*256+t*128+p
    MT16 = np.ascontiguousarray(
        MT16.reshape(4, 2, 128, D).transpose(0, 2, 1, 3).reshape(D, D))
    mk = np.asarray(mask, dtype=np.int32).astype(np.float32)
    maps = []
    for i in range(NCORES):
        xs = x16[i * BL:(i + 1) * BL]
        xt = xs[:, :ST, :].transpose(0, 2, 1)
        xt = np.ascontiguousarray(
            xt.reshape(BL, 4, 2, 128, ST).transpose(0, 1, 3, 2, 4)
            .reshape(BL, D, ST))
        # xn host-packed: row p holds chunks i=0..NN-1 back to back, so
        # each partition line is a single NN*D*2 = 10 KB contiguous run
        xn = xs[:, ST:, :].reshape(BL, NN, P, D).transpose(0, 2, 1, 3)
        xn = np.ascontiguousarray(xn.reshape(BL, P, NN * D))
        mkc = mk[i * BL:(i + 1) * BL].reshape(BL, J, P).transpose(2, 0, 1)
        maps.append({
            "xt16": xt,
            "xn16": xn,
            "yT16": np.ascontiguousarray(y16[i * BL:(i + 1) * BL].T),
            "MT16": MT16,
            "maskT": np.ascontiguousarray(mkc),
        })
    return maps


def kernel(x, y, mask, M, **_ignored):
    from concourse.bass_utils import run_bass_kernel_spmd

    nc = _get_nc()
    res = run_bass_kernel_spmd(nc, _in_maps(x, y, mask, M),
                               core_ids=list(range(NCORES)))
    out = np.concatenate([res.results[i]["out"] for i in range(NCORES)],
                         axis=0)
    return out.astype(np.float32)


# revision 15
# speedup vs baseline: 1.0986x; 1.0986x over previous
"""Trainium2 Bass kernel for masked attention scoring (sparse_attention).

Computes, per batch b:
    proj = y @ M^T                      # [B, D]
    eij  = tanh(einsum('bsd,bd->bs', x, proj))
    a    = exp(eij) * mask
    a    = a / (sum_s a + EPS)

Sharding: data-parallel over batch B=32 across 8 NeuronCores (4 batches
per core). M is replicated; all reductions stay local per shard.

Design (memory-bound; the x f16 stream at ~420 GB/s/core is the
roofline; instruction fetch rides DMA engine 0, so PE-instruction
bytes executed during the stream directly slow that engine and delay
every DMA-completion semaphore -> keep instruction count low):
  - x splits into a host-TRANSPOSED part xt (s-chunks 0..NT-1, [e, s]
    with d-rows pairwise interleaved, 5.6 KB descriptors) consumed by
    TensorE as 128x128 LDWEIGHTS tiles against a projT column (eij
    accumulates in PSUM on 128 partitions), and a natural-layout part
    xn (chunks NT..15, host-packed so each partition row is one 10 KB
    descriptor) for the cheap-in-instructions DVE STT / DVE-mul + ACT
    Copy-accum path.
  - The PE x-accumulation runs per batch in two phases (one per e-chunk
    half / DMA piece), each phase a contiguous 4-matmul PSUM group per
    column in its own bank; ACT copy + single-PSUM-operand DVE add
    merge the banks.
  - Per-batch epilogue runs INLINE (tanh+exp, fused DVE mask-mul +
    row-accum -> au / cs), then sum+broadcast in ONE matmul
    (lhsT=ones[P,J] replicates the total onto J partitions), +eps,
    reciprocal, PE transpose of unnormalized au, normalization as a
    per-partition ACT scale on the final copy.  Out-DMAs ride the
    second HWDGE ring (scalar engine) so they skip the x backlog on
    the sync-ring FIFO.
  - The last batch's phase-B xt piece and epilogue are split at
    j=JSPLIT so nearly all tail work completes before the final
    ~400 KB piece lands; mask/M/y/xn stream before xt so the last
    batch's dependent chain is the shortest one.
"""

import os
import sys

import numpy as np

for _p in ("/opt/trn_rl_repo",):
    if os.path.isdir(_p) and _p not in sys.path:
        sys.path.insert(0, _p)

B, S, D = 32, 2048, 1024
NCORES = 8
BL = B // NCORES        # batches per core
P = 128                 # SBUF partitions
J = S // P              # 16 s-chunks of 128 per batch
DC = D // P             # 8 e-chunks of 128
NT = 11                 # PE-path s-chunks per batch (j = 0..NT-1)
NN = J - NT             # natural-path s-chunks (j = NT..J-1)
N_STT = 2               # of NN: chunks done as fused STT on DVE
ST = NT * P
SN = NN * P
JSPLIT = 8              # last batch: phase-B piece/epilogue split (j)
EPS = 1e-7

_CACHE = {}


def _build():
    import concourse.bacc as bacc
    import concourse.bass as bass_mod
    import concourse.tile as tile
    from concourse import mybir
    from concourse.masks import make_identity

    f32 = mybir.dt.float32
    f16 = mybir.dt.float16

    nc = bacc.Bacc("TRN2", target_bir_lowering=False, debug=False,
                   num_devices=NCORES)

    xt_ext = nc.dram_tensor("xt16", [BL, D, ST], f16, kind="ExternalInput").ap()
    xn_ext = nc.dram_tensor("xn16", [BL, P, NN * D], f16,
                            kind="ExternalInput").ap()
    y_ext = nc.dram_tensor("yT16", [P, DC * BL], f16,
                           kind="ExternalInput").ap()
    m_ext = nc.dram_tensor("MT16", [D, D], f16, kind="ExternalInput").ap()
    mk_ext = nc.dram_tensor("maskT", [P, BL, J], f32, kind="ExternalInput").ap()
    out_ext = nc.dram_tensor("out", [BL, S], f32, kind="ExternalOutput").ap()

    with tile.TileContext(nc) as tc:
        with (
            tc.tile_pool(name="consts", bufs=1) as consts,
            tc.tile_pool(name="psum_proj", bufs=1, space="PSUM") as psum_proj,
            tc.tile_pool(name="psum_eij", bufs=1, space="PSUM") as psum_eij,
            tc.tile_pool(name="psum_eij2", bufs=1, space="PSUM") as psum_eij2,
            tc.tile_pool(name="psum_pb", bufs=1, space="PSUM") as psum_pb,
            tc.tile_pool(name="psum_small", bufs=1, space="PSUM") as psum_small,
            tc.tile_pool(name="scr", bufs=6) as scr_pool,
        ):
            identity16 = consts.tile([P, P], f16)
            make_identity(nc, identity16)
            identity32 = consts.tile([P, P], f32)
            make_identity(nc, identity32)
            ones_pj = consts.tile([P, J], f32)
            nc.vector.memset(ones_pj, 1.0)

            # ---- M^T ships pre-transposed + pairwise row-interleaved:
            # host row q=dc2*256+2p+t holds MT row d=dc2*256+t*128+p, so
            # each (p,dc2) partition line is a 4KB contiguous run and the
            # contraction mapping stays d = dc*128+p with dc=2*dc2+t.
            mtsb = consts.tile([P, DC // 2, 2, D], f16)
            m_src = m_ext.rearrange("(dc2 p two) e -> p dc2 two e",
                                    p=P, two=2)
            nc.sync.dma_start(out=mtsb, in_=m_src)

            # warm the PE clock (1.2 -> 2.4 GHz needs ~3us sustained)
            warm_ps = psum_small.tile([P, P], f16, tag="small")
            for _ in range(12):
                nc.tensor.transpose(warm_ps, identity16, identity16)

            # ---- small inputs ----
            yT = consts.tile([P, DC, BL], f16)
            nc.sync.dma_start(
                out=yT, in_=y_ext.rearrange("p (dc b) -> p dc b", b=BL))
            mask_all = consts.tile([P, BL, J], f32)
            nc.sync.dma_start(out=mask_all, in_=mk_ext)

            # ---- x DMAs: xn (slow consumers) first, xt last ----
            xt_tiles = []
            xn_tiles = []
            for b in range(BL):
                xt_tiles.append(consts.tile([P, DC // 2, 2, ST], f16,
                                            name=f"xt{b}"))
                xn_tiles.append(consts.tile([P, NN, D], f16, name=f"xn{b}"))
            for b in range(BL):
                # host-packed: each partition row is one 10 KB run
                nc.sync.dma_start(
                    out=xn_tiles[b],
                    in_=xn_ext[b].rearrange("p (i e) -> p i e", i=NN))
            # xt rows are 2 interleaved d-rows = 5.6KB descriptors; split
            # by e-chunk halves (phases).  NEVER slice along s: that
            # shrinks descriptors below ~2KB and the tail then drains at
            # ~1/3 line rate.  The last batch instead splits its phase-B
            # piece at dc2 granularity so the final piece is 1.4 MB.
            for b in range(BL):
                xt_src = xt_ext[b].rearrange("(dc2 p two) s -> p dc2 two s",
                                             p=P, two=2)
                nc.sync.dma_start(out=xt_tiles[b][:, 0:2, :, :],
                                  in_=xt_src[:, 0:2, :, :])
                if b < BL - 1:
                    nc.sync.dma_start(out=xt_tiles[b][:, 2:, :, :],
                                      in_=xt_src[:, 2:, :, :])
                else:
                    nc.sync.dma_start(out=xt_tiles[b][:, 2:3, :, :],
                                      in_=xt_src[:, 2:3, :, :])
                    nc.sync.dma_start(out=xt_tiles[b][:, 3:, :, :],
                                      in_=xt_src[:, 3:, :, :])

            # ---- proj[b, e] = sum_d y[b, d] * M[e, d]  (PSUM f32) ----
            proj_ps = psum_proj.tile([BL, D], f32)
            for dc in range(DC):
                for eh in range(2):
                    nc.tensor.matmul(
                        proj_ps[:, eh * 512:(eh + 1) * 512],
                        lhsT=yT[:, dc, :],
                        rhs=mtsb[:, dc // 2, dc % 2,
                                 eh * 512:(eh + 1) * 512],
                        start=(dc == 0),
                        stop=(dc == DC - 1),
                    )
            proj_sb = consts.tile([BL, D], f16)
            nc.scalar.copy(proj_sb[:, 0:512], proj_ps[:, 0:512])
            nc.scalar.copy(proj_sb[:, 512:], proj_ps[:, 512:])

            # ---- projT[p, ec, b] = proj[b, ec*128+p] via PE transposes ----
            projT = consts.tile([P, DC, BL], f16)
            for ec in range(DC):
                tp_ps = psum_small.tile([P, BL], f16, tag="ptr", bufs=1)
                nc.tensor.transpose(
                    tp_ps, proj_sb[:, ec * P:(ec + 1) * P],
                    identity16[:BL, :BL])
                nc.scalar.copy(projT[:, ec, :], tp_ps)

            # ---- broadcast proj rows across partitions (DVE path) ----
            projbc = []
            for b in range(BL):
                sel = consts.tile([BL, P], f16, name=f"sel{b}")
                nc.gpsimd.memset(sel, 0.0)
                nc.gpsimd.affine_select(
                    out=sel, in_=sel,
                    compare_op=mybir.AluOpType.not_equal,
                    fill=1.0, base=-b,
                    pattern=[[0, P]], channel_multiplier=1)
                pb = consts.tile([P, D], f16, name=f"projbc{b}")
                for eh in range(2):
                    pb_ps = psum_pb.tile([P, 512], f32, tag="pbps")
                    nc.tensor.matmul(
                        pb_ps,
                        lhsT=sel,
                        rhs=proj_sb[:, eh * 512:(eh + 1) * 512],
                        start=True, stop=True)
                    if eh == 0:
                        nc.vector.tensor_copy(pb[:, eh * 512:(eh + 1) * 512],
                                              pb_ps)
                    else:
                        nc.scalar.copy(pb[:, eh * 512:(eh + 1) * 512], pb_ps)
                projbc.append(pb)

            # ---- main pass tiles ----
            eij_a = consts.tile([P, BL, NT], f32)
            eij_all = consts.tile([P, BL, J], f32)
            au = consts.tile([P, BL, J], f32)
            cs = consts.tile([P, BL], f32)
            csb = consts.tile([P, 1], f32)
            th = consts.tile([P, BL, J], f32)
            ex = consts.tile([P, BL, J], f32)

            # ---- main pass: DVE/ACT on xn chunks (writes eij_all[NT:]) ----
            for b in range(BL):
                xn = xn_tiles[b]
                for i in range(N_STT):
                    scr = scr_pool.tile([P, D], f16, tag="scr")
                    nc.vector.scalar_tensor_tensor(
                        out=scr,
                        in0=xn[:, i, :],
                        scalar=1.0,
                        in1=projbc[b],
                        op0=mybir.AluOpType.mult,
                        op1=mybir.AluOpType.mult,
                        accum_out=eij_all[:, b, NT + i:NT + i + 1],
                    )
                i = N_STT
                while i < NN:
                    w = min(2, NN - i)
                    scr2 = scr_pool.tile([P, 2, D], f16, tag="scr2", bufs=4)
                    pbc2 = bass_mod.AP(
                        tensor=projbc[b].tensor,
                        offset=projbc[b].offset,
                        ap=[projbc[b].ap[0], [0, w]] + projbc[b].ap[1:])
                    nc.vector.tensor_mul(scr2[:, 0:w, :],
                                         xn[:, i:i + w, :], pbc2)
                    for k in range(w):
                        dump = scr_pool.tile([P, D], f16, tag="dump", bufs=4)
                        nc.scalar.activation(
                            dump, scr2[:, k, :],
                            mybir.ActivationFunctionType.Copy,
                            accum_out=eij_all[:, b, NT + i + k:NT + i + k + 1])
                    i += w

            # ---- per-batch epilogue pieces ----
            def epilogue(b, j0, j1, m0, m1, ps2, cs_col):
                # merge PE phase banks for columns m0..m1, then
                # tanh/exp/mask-mul+accum over columns j0..j1
                if m1 > m0:
                    nc.vector.tensor_add(eij_all[:, b, m0:m1],
                                         eij_a[:, b, m0:m1], ps2[:, b, m0:m1])
                nc.scalar.activation(th[:, b, j0:j1], eij_all[:, b, j0:j1],
                                     mybir.ActivationFunctionType.Tanh)
                nc.scalar.activation(ex[:, b, j0:j1], th[:, b, j0:j1],
                                     mybir.ActivationFunctionType.Exp)
                nc.vector.scalar_tensor_tensor(
                    out=au[:, b, j0:j1],
                    in0=ex[:, b, j0:j1],
                    scalar=1.0,
                    in1=mask_all[:, b, j0:j1],
                    op0=mybir.AluOpType.mult,
                    op1=mybir.AluOpType.mult,
                    accum_out=cs_col,
                )

            def finishing(b, cs_cols):
                # total-sum replicated onto J partitions in one matmul
                tot_ps = psum_small.tile([J, 1], f32, tag="small")
                for i, col in enumerate(cs_cols):
                    nc.tensor.matmul(tot_ps, lhsT=ones_pj, rhs=col,
                                     start=(i == 0),
                                     stop=(i == len(cs_cols) - 1))
                at_ps = psum_small.tile([J, P], f32, tag="attr")
                nc.tensor.transpose(at_ps, au[:, b, :], identity32)
                rec = consts.tile([J, 1], f32, name=f"rec{b}")
                nc.vector.tensor_scalar_add(rec, tot_ps, EPS)
                nc.vector.reciprocal(rec, rec)
                an_t = consts.tile([J, P], f32, name=f"ant{b}")
                nc.scalar.activation(an_t, at_ps,
                                     mybir.ActivationFunctionType.Copy,
                                     scale=rec)
                # out-DMAs ride the scalar HWDGE ring: independent FIFO,
                # and the issue directly follows the scale-copy in the
                # ACT queue (no cross-engine hop).
                nc.scalar.dma_start(
                    out=out_ext[b].rearrange("(j p) -> j p", p=P),
                    in_=an_t)

            # ---- main pass: PE on xt chunks; epilogue inline ----
            for b in range(BL):
                xt = xt_tiles[b]
                eij_ps = psum_eij.tile([P, BL, NT], f32, tag="eijA")
                eij_ps2 = psum_eij2.tile([P, BL, NT], f32, tag="eijB")
                # phase A (e-chunks 0..3), bank A; contiguous group per col
                for j in range(NT):
                    for ec in range(4):
                        nc.tensor.matmul(
                            eij_ps[:, b, j:j + 1],
                            lhsT=xt[:, ec // 2, ec % 2, j * P:(j + 1) * P],
                            rhs=projT[:, ec, b:b + 1],
                            start=(ec == 0),
                            stop=(ec == 3),
                        )
                nc.scalar.copy(eij_a[:, b, :], eij_ps[:, b, :])

                def phase_b(j0, j1):
                    for j in range(j0, j1):
                        for ec in range(4, 8):
                            nc.tensor.matmul(
                                eij_ps2[:, b, j:j + 1],
                                lhsT=xt[:, ec // 2, ec % 2,
                                        j * P:(j + 1) * P],
                                rhs=projT[:, ec, b:b + 1],
                                start=(ec == 4),
                                stop=(ec == 7),
                            )

                phase_b(0, NT)
                epilogue(b, 0, J, 0, NT, eij_ps2, cs[:, b:b + 1])
                finishing(b, [cs[:, b:b + 1]])

    nc.compile()
    return nc


def _get_nc():
    if "nc" not in _CACHE:
        _CACHE["nc"] = _build()
    return _CACHE["nc"]


def _in_maps(x, y, mask, M):
    x16 = np.asarray(x, dtype=np.float32).astype(np.float16)
    y16 = np.asarray(y, dtype=np.float32).astype(np.float16)
    MT16 = np.asarray(M, dtype=np.float32).astype(np.float16).T
    # interleave d-rows pairwise: row q=dc2*256+2p+t holds d=dc2*256+t*128+p
    MT16 = np.ascontiguousarray(
        MT16.reshape(4, 2, 128, D).transpose(0, 2, 1, 3).reshape(D, D))
    mk = np.asarray(mask, dtype=np.int32).astype(np.float32)
    maps = []
    for i in range(NCORES):
        xs = x16[i * BL:(i + 1) * BL]
        xt = xs[:, :ST, :].transpose(0, 2, 1)
        xt = np.ascontiguousarray(
            xt.reshape(BL, 4, 2, 128, ST).transpose(0, 1, 3, 2, 4)
            .reshape(BL, D, ST))
        # xn host-packed: row p holds chunks i=0..NN-1 back to back, so
        # each partition line is a single NN*D*2 = 10 KB contiguous run
        xn = xs[:, ST:, :].reshape(BL, NN, P, D).transpose(0, 2, 1, 3)
        xn = np.ascontiguousarray(xn.reshape(BL, P, NN * D))
        ys = y16[i * BL:(i + 1) * BL]
        yT = np.ascontiguousarray(
            ys.T.reshape(DC, P, BL).transpose(1, 0, 2).reshape(P, DC * BL))
        mkc = mk[i * BL:(i + 1) * BL].reshape(BL, J, P).transpose(2, 0, 1)
        maps.append({
            "xt16": xt,
            "xn16": xn,
            "yT16": yT,
            "MT16": MT16,
            "maskT": np.ascontiguousarray(mkc),
        })
    return maps


def kernel(x, y, mask, M, **_ignored):
    from concourse.bass_utils import run_bass_kernel_spmd

    nc = _get_nc()
    res = run_bass_kernel_spmd(nc, _in_maps(x, y, mask, M),
                               core_ids=list(range(NCORES)))
    out = np.concatenate([res.results[i]["out"] for i in range(NCORES)],
                         axis=0)
    return out.astype(np.float32)


# revision 20
# speedup vs baseline: 1.1267x; 1.0256x over previous
"""Trainium2 Bass kernel for masked attention scoring (sparse_attention).

Computes, per batch b:
    proj = y @ M^T                      # [B, D]
    eij  = tanh(einsum('bsd,bd->bs', x, proj))
    a    = exp(eij) * mask
    a    = a / (sum_s a + EPS)

Sharding: data-parallel over batch B=32 across 8 NeuronCores (4 batches
per core). M is replicated; all reductions stay local per shard.

Design (memory-bound; the x f16 stream at ~420 GB/s/core is the
roofline; instruction fetch rides DMA engine 0, so PE-instruction
bytes executed during the stream directly slow that engine and delay
every DMA-completion semaphore -> keep instruction count low):
  - x splits into a host-TRANSPOSED part xt (s-chunks 0..NT-1, [e, s]
    with d-rows pairwise interleaved, 5.6 KB descriptors) consumed by
    TensorE as 128x128 LDWEIGHTS tiles against a projT column (eij
    accumulates in PSUM on 128 partitions), and a natural-layout part
    xn (chunks NT..15, host-packed so each partition row is one 10 KB
    descriptor) for the cheap-in-instructions DVE STT / DVE-mul + ACT
    Copy-accum path.
  - The PE x-accumulation runs per batch in two phases (one per e-chunk
    half / DMA piece), each phase a contiguous 4-matmul PSUM group per
    column in its own bank; ACT copy + single-PSUM-operand DVE add
    merge the banks.
  - Per-batch epilogue runs INLINE (tanh+exp, fused DVE mask-mul +
    row-accum -> au / cs), then sum+broadcast in ONE matmul
    (lhsT=ones[P,J] replicates the total onto J partitions), +eps,
    reciprocal, PE transpose of unnormalized au, normalization as a
    per-partition ACT scale on the final copy.  Out-DMAs ride the
    second HWDGE ring (scalar engine) so they skip the x backlog on
    the sync-ring FIFO.
  - The last batch's phase-B xt piece and epilogue are split at
    j=JSPLIT so nearly all tail work completes before the final
    ~400 KB piece lands; mask/M/y/xn stream before xt so the last
    batch's dependent chain is the shortest one.
"""

import os
import sys

import numpy as np

for _p in ("/opt/trn_rl_repo",):
    if os.path.isdir(_p) and _p not in sys.path:
        sys.path.insert(0, _p)

B, S, D = 32, 2048, 1024
NCORES = 8
BL = B // NCORES        # batches per core
P = 128                 # SBUF partitions
J = S // P              # 16 s-chunks of 128 per batch
DC = D // P             # 8 e-chunks of 128
NT = 11                 # PE-path s-chunks per batch (j = 0..NT-1)
NN = J - NT             # natural-path s-chunks (j = NT..J-1)
N_STT = 2               # of NN: chunks done as fused STT on DVE
ST = NT * P
SN = NN * P
JSPLIT = 8              # last batch: phase-B piece/epilogue split (j)
EPS = 1e-7

_CACHE = {}


def _build():
    import concourse.bacc as bacc
    import concourse.bass as bass_mod
    import concourse.tile as tile
    from concourse import mybir
    from concourse.masks import make_identity

    f32 = mybir.dt.float32
    f16 = mybir.dt.float16

    nc = bacc.Bacc("TRN2", target_bir_lowering=False, debug=False,
                   num_devices=NCORES)

    xt_ext = nc.dram_tensor("xt16", [BL, D, ST], f16, kind="ExternalInput").ap()
    xn_ext = nc.dram_tensor("xn16", [BL, P, NN * D], f16,
                            kind="ExternalInput").ap()
    y_ext = nc.dram_tensor("yT16", [P, DC * BL], f16,
                           kind="ExternalInput").ap()
    m_ext = nc.dram_tensor("MT16", [D, D], f16, kind="ExternalInput").ap()
    mk_ext = nc.dram_tensor("maskT", [P, BL, J], f32, kind="ExternalInput").ap()
    out_ext = nc.dram_tensor("out", [BL, S], f32, kind="ExternalOutput").ap()

    with tile.TileContext(nc) as tc:
        with (
            tc.tile_pool(name="consts", bufs=1) as consts,
            tc.tile_pool(name="psum_proj", bufs=1, space="PSUM") as psum_proj,
            tc.tile_pool(name="psum_eij", bufs=1, space="PSUM") as psum_eij,
            tc.tile_pool(name="psum_eij2", bufs=1, space="PSUM") as psum_eij2,
            tc.tile_pool(name="psum_eij3", bufs=1, space="PSUM") as psum_eij3,
            tc.tile_pool(name="psum_pb", bufs=1, space="PSUM") as psum_pb,
            tc.tile_pool(name="psum_small", bufs=1, space="PSUM") as psum_small,
            tc.tile_pool(name="scr", bufs=6) as scr_pool,
        ):
            identity16 = consts.tile([P, P], f16)
            make_identity(nc, identity16)
            identity32 = consts.tile([P, P], f32)
            make_identity(nc, identity32)
            ones_pj = consts.tile([P, J], f32)
            nc.vector.memset(ones_pj, 1.0)

            # ---- M^T ships pre-transposed + pairwise row-interleaved:
            # host row q=dc2*256+2p+t holds MT row d=dc2*256+t*128+p, so
            # each (p,dc2) partition line is a 4KB contiguous run and the
            # contraction mapping stays d = dc*128+p with dc=2*dc2+t.
            mtsb = consts.tile([P, DC // 2, 2, D], f16)
            m_src = m_ext.rearrange("(dc2 p two) e -> p dc2 two e",
                                    p=P, two=2)
            nc.sync.dma_start(out=mtsb, in_=m_src)

            # warm the PE clock (1.2 -> 2.4 GHz needs ~3us sustained)
            warm_ps = psum_small.tile([P, P], f16, tag="small")
            for _ in range(12):
                nc.tensor.transpose(warm_ps, identity16, identity16)

            # ---- small inputs ----
            yT = consts.tile([P, DC, BL], f16)
            nc.sync.dma_start(
                out=yT, in_=y_ext.rearrange("p (dc b) -> p dc b", b=BL))
            mask_all = consts.tile([P, BL, J], f32)
            nc.sync.dma_start(out=mask_all, in_=mk_ext)

            # ---- x DMAs: xn (slow consumers) first, xt last ----
            xt_tiles = []
            xn_tiles = []
            for b in range(BL):
                xt_tiles.append(consts.tile([P, DC // 2, 2, ST], f16,
                                            name=f"xt{b}"))
                xn_tiles.append(consts.tile([P, NN, D], f16, name=f"xn{b}"))
            for b in range(BL):
                # host-packed: each partition row is one 10 KB run
                nc.sync.dma_start(
                    out=xn_tiles[b],
                    in_=xn_ext[b].rearrange("p (i e) -> p i e", i=NN))
            # xt rows are 2 interleaved d-rows = 5.6KB descriptors; split
            # by e-chunk halves (phases).  NEVER slice along s: that
            # shrinks descriptors below ~2KB and the tail then drains at
            # ~1/3 line rate.  The last batch instead splits its phase-B
            # piece at dc2 granularity so the final piece is 1.4 MB.
            for b in range(BL):
                xt_src = xt_ext[b].rearrange("(dc2 p two) s -> p dc2 two s",
                                             p=P, two=2)
                nc.sync.dma_start(out=xt_tiles[b][:, 0:2, :, :],
                                  in_=xt_src[:, 0:2, :, :])
                if b < BL - 1:
                    nc.sync.dma_start(out=xt_tiles[b][:, 2:, :, :],
                                      in_=xt_src[:, 2:, :, :])
                else:
                    nc.sync.dma_start(out=xt_tiles[b][:, 2:3, :, :],
                                      in_=xt_src[:, 2:3, :, :])
                    nc.sync.dma_start(out=xt_tiles[b][:, 3:, :, :],
                                      in_=xt_src[:, 3:, :, :])

            # ---- proj[b, e] = sum_d y[b, d] * M[e, d]  (PSUM f32) ----
            # two e-halves through ONE [BL, 512] bank (re-used serially)
            # so the third eij phase bank below fits in the 8-bank budget
            proj_sb = consts.tile([BL, D], f16)
            for eh in range(2):
                proj_ps = psum_proj.tile([BL, 512], f32, tag="proj")
                for dc in range(DC):
                    nc.tensor.matmul(
                        proj_ps,
                        lhsT=yT[:, dc, :],
                        rhs=mtsb[:, dc // 2, dc % 2,
                                 eh * 512:(eh + 1) * 512],
                        start=(dc == 0),
                        stop=(dc == DC - 1),
                    )
                nc.scalar.copy(proj_sb[:, eh * 512:(eh + 1) * 512], proj_ps)

            # ---- projT[p, ec, b] = proj[b, ec*128+p] via PE transposes ----
            projT = consts.tile([P, DC, BL], f16)
            for ec in range(DC):
                tp_ps = psum_small.tile([P, BL], f16, tag="ptr", bufs=1)
                nc.tensor.transpose(
                    tp_ps, proj_sb[:, ec * P:(ec + 1) * P],
                    identity16[:BL, :BL])
                nc.scalar.copy(projT[:, ec, :], tp_ps)

            # ---- broadcast proj rows across partitions (DVE path) ----
            projbc = []
            for b in range(BL):
                sel = consts.tile([BL, P], f16, name=f"sel{b}")
                nc.gpsimd.memset(sel, 0.0)
                nc.gpsimd.affine_select(
                    out=sel, in_=sel,
                    compare_op=mybir.AluOpType.not_equal,
                    fill=1.0, base=-b,
                    pattern=[[0, P]], channel_multiplier=1)
                pb = consts.tile([P, D], f16, name=f"projbc{b}")
                for eh in range(2):
                    pb_ps = psum_pb.tile([P, 512], f32, tag="pbps")
                    nc.tensor.matmul(
                        pb_ps,
                        lhsT=sel,
                        rhs=proj_sb[:, eh * 512:(eh + 1) * 512],
                        start=True, stop=True)
                    if eh == 0:
                        nc.vector.tensor_copy(pb[:, eh * 512:(eh + 1) * 512],
                                              pb_ps)
                    else:
                        nc.scalar.copy(pb[:, eh * 512:(eh + 1) * 512], pb_ps)
                projbc.append(pb)

            # ---- main pass tiles ----
            eij_a = consts.tile([P, BL, NT], f32)
            eij_all = consts.tile([P, BL, J], f32)
            eij_b3 = consts.tile([P, NT], f32)
            au = consts.tile([P, BL, J], f32)
            cs = consts.tile([P, BL], f32)
            th = consts.tile([P, BL, J], f32)
            ex = consts.tile([P, BL, J], f32)

            # ---- main pass: DVE/ACT on xn chunks (writes eij_all[NT:]) ----
            for b in range(BL):
                xn = xn_tiles[b]
                for i in range(N_STT):
                    scr = scr_pool.tile([P, D], f16, tag="scr")
                    nc.vector.scalar_tensor_tensor(
                        out=scr,
                        in0=xn[:, i, :],
                        scalar=1.0,
                        in1=projbc[b],
                        op0=mybir.AluOpType.mult,
                        op1=mybir.AluOpType.mult,
                        accum_out=eij_all[:, b, NT + i:NT + i + 1],
                    )
                i = N_STT
                while i < NN:
                    w = min(2, NN - i)
                    scr2 = scr_pool.tile([P, 2, D], f16, tag="scr2", bufs=4)
                    pbc2 = bass_mod.AP(
                        tensor=projbc[b].tensor,
                        offset=projbc[b].offset,
                        ap=[projbc[b].ap[0], [0, w]] + projbc[b].ap[1:])
                    nc.vector.tensor_mul(scr2[:, 0:w, :],
                                         xn[:, i:i + w, :], pbc2)
                    for k in range(w):
                        dump = scr_pool.tile([P, D], f16, tag="dump", bufs=4)
                        nc.scalar.activation(
                            dump, scr2[:, k, :],
                            mybir.ActivationFunctionType.Copy,
                            accum_out=eij_all[:, b, NT + i + k:NT + i + k + 1])
                    i += w

            # ---- per-batch epilogue pieces ----
            def tanh_exp(b, j0, j1, src):
                nc.scalar.activation(th[:, b, j0:j1], src,
                                     mybir.ActivationFunctionType.Tanh)
                nc.scalar.activation(ex[:, b, j0:j1], th[:, b, j0:j1],
                                     mybir.ActivationFunctionType.Exp)

            def mask_accum(b, cs_col):
                nc.vector.scalar_tensor_tensor(
                    out=au[:, b, :],
                    in0=ex[:, b, :],
                    scalar=1.0,
                    in1=mask_all[:, b, :],
                    op0=mybir.AluOpType.mult,
                    op1=mybir.AluOpType.mult,
                    accum_out=cs_col,
                )

            def finishing(b, cs_cols):
                # total-sum replicated onto J partitions in one matmul
                tot_ps = psum_small.tile([J, 1], f32, tag="small")
                for i, col in enumerate(cs_cols):
                    nc.tensor.matmul(tot_ps, lhsT=ones_pj, rhs=col,
                                     start=(i == 0),
                                     stop=(i == len(cs_cols) - 1))
                at_ps = psum_small.tile([J, P], f32, tag="attr")
                nc.tensor.transpose(at_ps, au[:, b, :], identity32)
                rec = consts.tile([J, 1], f32, name=f"rec{b}")
                nc.vector.tensor_scalar_add(rec, tot_ps, EPS)
                nc.vector.reciprocal(rec, rec)
                an_t = consts.tile([J, P], f32, name=f"ant{b}")
                nc.scalar.activation(an_t, at_ps,
                                     mybir.ActivationFunctionType.Copy,
                                     scale=rec)
                # out-DMAs ride the scalar HWDGE ring: independent FIFO,
                # and the issue directly follows the scale-copy in the
                # ACT queue (no cross-engine hop).
                nc.scalar.dma_start(
                    out=out_ext[b].rearrange("(j p) -> j p", p=P),
                    in_=an_t)

            # ---- main pass: PE on xt chunks; epilogue inline.  Batch
            # b's finishing PE ops (tot matmul + au transpose) wait on
            # the DVE chain, so they are DEFERRED until after batch
            # b+1's phase-A matmuls - otherwise they head-of-line block
            # the in-order PE queue and the pipeline accrues ~2 us of
            # lag per batch.
            pending = None
            for b in range(BL):
                xt = xt_tiles[b]
                eij_ps = psum_eij.tile([P, BL, NT], f32, tag="eijA")
                eij_ps2 = psum_eij2.tile([P, BL, NT], f32, tag="eijB")

                def phase(ps, e0, e1, col3=False):
                    for j in range(NT):
                        for ec in range(e0, e1):
                            nc.tensor.matmul(
                                ps[:, j:j + 1] if col3 else ps[:, b, j:j + 1],
                                lhsT=xt[:, ec // 2, ec % 2,
                                        j * P:(j + 1) * P],
                                rhs=projT[:, ec, b:b + 1],
                                start=(ec == e0),
                                stop=(ec == e1 - 1),
                            )

                # phase A (e-chunks 0..3), bank A; contiguous group per col
                phase(eij_ps, 0, 4)
                nc.scalar.copy(eij_a[:, b, :], eij_ps[:, b, :])
                if pending is not None:
                    finishing(*pending)
                    pending = None
                if b < BL - 1:
                    phase(eij_ps2, 4, 8)
                    nc.vector.tensor_add(eij_all[:, b, 0:NT],
                                         eij_a[:, b, :], eij_ps2[:, b, :])
                    tanh_exp(b, 0, J, eij_all[:, b, :])
                    mask_accum(b, cs[:, b:b + 1])
                    pending = (b, [cs[:, b:b + 1]])
                else:
                    # last batch: three PE phases matching its three DMA
                    # pieces; everything except the final 22 matmuls and
                    # the tail chain runs while the stream still flows
                    tanh_exp(b, NT, J, eij_all[:, b, NT:J])
                    phase(eij_ps2, 4, 6)
                    nc.vector.tensor_add(eij_all[:, b, 0:NT],
                                         eij_a[:, b, :], eij_ps2[:, b, :])
                    eij_ps3 = psum_eij3.tile([P, NT], f32, tag="eijC")
                    phase(eij_ps3, 6, 8, col3=True)
                    nc.vector.tensor_add(eij_b3, eij_all[:, b, 0:NT],
                                         eij_ps3)
                    tanh_exp(b, 0, NT, eij_b3)
                    mask_accum(b, cs[:, b:b + 1])
                    finishing(b, [cs[:, b:b + 1]])

    nc.compile()
    return nc


def _get_nc():
    if "nc" not in _CACHE:
        _CACHE["nc"] = _build()
    return _CACHE["nc"]


def _in_maps(x, y, mask, M):
    x16 = np.asarray(x, dtype=np.float32).astype(np.float16)
    y16 = np.asarray(y, dtype=np.float32).astype(np.float16)
    MT16 = np.asarray(M, dtype=np.float32).astype(np.float16).T
    # interleave d-rows pairwise: row q=dc2*256+2p+t holds d=dc2*256+t*128+p
    MT16 = np.ascontiguousarray(
        MT16.reshape(4, 2, 128, D).transpose(0, 2, 1, 3).reshape(D, D))
    mk = np.asarray(mask, dtype=np.int32).astype(np.float32)
    maps = []
    for i in range(NCORES):
        xs = x16[i * BL:(i + 1) * BL]
        xt = xs[:, :ST, :].transpose(0, 2, 1)
        xt = np.ascontiguousarray(
            xt.reshape(BL, 4, 2, 128, ST).transpose(0, 1, 3, 2, 4)
            .reshape(BL, D, ST))
        # xn host-packed: row p holds chunks i=0..NN-1 back to back, so
        # each partition line is a single NN*D*2 = 10 KB contiguous run
        xn = xs[:, ST:, :].reshape(BL, NN, P, D).transpose(0, 2, 1, 3)
        xn = np.ascontiguousarray(xn.reshape(BL, P, NN * D))
        ys = y16[i * BL:(i + 1) * BL]
        yT = np.ascontiguousarray(
            ys.T.reshape(DC, P, BL).transpose(1, 0, 2).reshape(P, DC * BL))
        mkc = mk[i * BL:(i + 1) * BL].reshape(BL, J, P).transpose(2, 0, 1)
        maps.append({
            "xt16": xt,
            "xn16": xn,
            "yT16": yT,
            "MT16": MT16,
            "maskT": np.ascontiguousarray(mkc),
        })
    return maps


def kernel(x, y, mask, M, **_ignored):
    from concourse.bass_utils import run_bass_kernel_spmd

    nc = _get_nc()
    res = run_bass_kernel_spmd(nc, _in_maps(x, y, mask, M),
                               core_ids=list(range(NCORES)))
    out = np.concatenate([res.results[i]["out"] for i in range(NCORES)],
                         axis=0)
    return out.astype(np.float32)
